# revision 48
# baseline (speedup 1.0000x reference)
"""Trainium2 Bass kernel for a binarized BasicBlock (2x bconv3x3 + BN +
residual hardtanh + channel shuffle), data-parallel over batch on 8 cores.

Self-contained: hardcodes shapes from the problem spec.
  x: (32, 256, 56, 56) f32 -> out: (32, 256, 56, 56) f32

Layout strategy:
- conv operands live in a zero/half-padded 58-wide u-domain layout
  [128, 59, 64] fp8 ({0,1} values, pads 0.5; the sign-domain correction is
  folded into the BN bias on host). conv = 9 accumulating matmuls per tile
  (3 fp8 DoubleRow vertical tap pairs + 3 singles); tiles are emitted in
  4-tile groups sharing a [128, 4, 512] PSUM allocation (all 8 banks in
  flight) so LDWEIGHTS amortizes over 4 matmuls per weight.
- the sign-critical path (everything feeding a binarize) is kept in f32
  with the exact op/rounding structure of the reference; bf16 appears only
  at DMA-feeding edges (outputs, pass-through idle-hi input quarter).
- engine assignment is microbenchmark-calibrated: DVE gets is_ge/clip/
  tensor_scalar (1.8us per image pass) and the two residual adds (3.4us);
  GpSimd gets immediate-scalar clips only (2.8us; per-partition pointers
  are catastrophically slow there); ACT drains PSUM (bn apply) and handles
  the bias-add passes. Whole-image single ops (fixed overhead dominates).
- move0 for the idle-lo half is folded into conv2's BN bias, and its
  binarize becomes a per-partition is_ge threshold (x >= -move0), which is
  bit-exactly equivalent to binarize(x + move0) in IEEE f32. The residual-2
  buffer A2 = [clip(v1)_lo ; raw x_idle_lo] makes u2 and the conv2 residual
  add single 128-partition ops.
- both channel_shuffles are free: host-permuted w2 channels + stride-4
  channel DMA writes.
"""

import numpy as np
import ml_dtypes

import concourse.bass as bass
import concourse.tile as tile
from concourse import bacc, mybir
from concourse import bass_utils

EPS = 1e-5
P = 128
H = W = 56
WP = 64          # padded row width (64 so vertical tap pairs are 16B-aligned
                 # apart, as fp8 DoubleRow requires)
RP = 59          # padded rows allocated (58 used + 1 spare for tail matmul reads)
IMGS_PER_CORE = 4
NCORES = 8
TF = 8 * WP      # matmul free size per tile = 512 (= one PSUM bank)

F32 = mybir.dt.float32
BF16 = mybir.dt.bfloat16
F8 = mybir.dt.float8e4
ALU = mybir.AluOpType
ACTF = mybir.ActivationFunctionType

_CACHE = {}


def _flat(ap3):
    return ap3.rearrange("p r c -> p (r c)")


def _build():
    nc = bacc.Bacc("TRN2", target_bir_lowering=False, debug=False)

    # x channels 0:192 (x_act + x_idle_lo) f32; channels 192:256 bf16
    xa_h = nc.dram_tensor("xs32", [IMGS_PER_CORE, 192, H, W], F32, kind="ExternalInput")
    xh_h = nc.dram_tensor("xs16", [IMGS_PER_CORE, 64, H, W], BF16, kind="ExternalInput")
    w1_h = nc.dram_tensor("w1m", [P, 9 * P], F8, kind="ExternalInput")
    w2_h = nc.dram_tensor("w2m", [P, 9 * P], F8, kind="ExternalInput")
    cst_h = nc.dram_tensor("cst", [P, 16], F32, kind="ExternalInput")
    out_h = nc.dram_tensor("out", [IMGS_PER_CORE, 2 * P, H, W], BF16, kind="ExternalOutput")

    xa_ap = xa_h.ap()
    xh_ap = xh_h.ap()

    def out_ch4(n, base_ch, r0=0, r1=H):
        # DRAM AP: channels base_ch, base_ch+4, ... of image n, rows r0:r1
        return bass.AP(
            tensor=out_h,
            offset=(n * 2 * P + base_ch) * H * W + r0 * W,
            ap=[[4 * H * W, 64], [1, (r1 - r0) * W]],
        )

    with tile.TileContext(nc) as tc:
        # persistent ping-pong buffers
        XA = [nc.alloc_sbuf_tensor(f"XA{i}", [P, H, W], F32).ap() for i in range(3)]
        A2 = [nc.alloc_sbuf_tensor(f"A2{i}", [P, H, W], F32).ap() for i in range(2)]
        XH = [nc.alloc_sbuf_tensor(f"XH{i}", [P, H, W], BF16).ap() for i in range(2)]
        V = [nc.alloc_sbuf_tensor(f"V{i}", [P, H, W], F32).ap() for i in range(2)]
        V2 = [nc.alloc_sbuf_tensor(f"V2{i}", [P, H, W], F32).ap() for i in range(2)]
        FO1 = [nc.alloc_sbuf_tensor(f"FO1{i}", [P, H, W], BF16).ap() for i in range(2)]
        OT2 = [nc.alloc_sbuf_tensor(f"OT2{i}", [P, H, W], BF16).ap() for i in range(2)]
        B1 = [nc.alloc_sbuf_tensor(f"B1{i}", [P, RP, WP], F8).ap() for i in range(2)]
        B2 = [nc.alloc_sbuf_tensor(f"B2{i}", [P, RP, WP], F8).ap() for i in range(2)]
        WS1 = nc.alloc_sbuf_tensor("WS1", [P, 9 * P], F8).ap()
        WS2 = nc.alloc_sbuf_tensor("WS2", [P, 9 * P], F8).ap()
        CST = nc.alloc_sbuf_tensor("CST", [P, 16], F32).ap()
        DUM = nc.alloc_sbuf_tensor("DUM", [P, 4], F32).ap()

        s1 = CST[:, 0:1]
        b1 = CST[:, 1:2]
        s2 = CST[:, 2:3]
        b2 = CST[:, 3:4]          # includes the +move0 fold on hi partitions
        beta_hi = CST[64:128, 4:5]  # move1_even
        u2thr = CST[:, 5:6]         # 0 on lo partitions, -move0[0:64] on hi
        cxh = CST[64:128, 8:9]      # move0[64:] + move1_odd
        beta_p1 = CST[64:128, 6:7]  # move1_even + 1
        beta_m1 = CST[64:128, 7:8]  # move1_even - 1

        # x_act image 0 first: it heads the critical path. Spread the head
        # descriptors across engine queues (dispatch is ~650ns each, serial
        # per queue).
        xa0 = XA[0]
        nc.sync.dma_start(out=xa0[:, 0:10], in_=xa_ap[0, 0:P, 0:10])
        nc.scalar.dma_start(out=WS1, in_=w1_h.ap())
        nc.sync.dma_start(out=xa0[:, 10:26], in_=xa_ap[0, 0:P, 10:26])
        nc.scalar.dma_start(out=xa0[:, 26:42], in_=xa_ap[0, 0:P, 26:42])
        nc.sync.dma_start(out=xa0[:, 42:56], in_=xa_ap[0, 0:P, 42:56])
        nc.scalar.dma_start(out=CST, in_=cst_h.ap())
        nc.scalar.dma_start(out=WS2, in_=w2_h.ap())

        # u-domain pads: 0.5 stands for binarized zero-padding. Interiors
        # are rewritten per image; pads never touched again, so only the
        # pad region is initialized (top row, bottom rows, side columns).
        for _b in (*B1, *B2):
            _f = _flat(_b)
            nc.gpsimd.memset(_f[:, 0:WP], 0.5)
            nc.gpsimd.memset(_f[:, 57 * WP:RP * WP], 0.5)
            nc.gpsimd.memset(_b[:, 1:57, 0:1], 0.5)
            nc.gpsimd.memset(_b[:, 1:57, 57:64], 0.5)

        with (
            tc.tile_pool(name="psum", bufs=2, space="PSUM") as psum_pool,
            tc.tile_pool(name="stage", bufs=3) as stage_pool,
        ):
            def emit_conv_group(ps, ws, bf, t0, ntiles):
                """One group of ntiles 8-row tiles starting at tile t0:
                3 DoubleRow vertical tap pairs + 3 single taps, weight-outer
                so each LDWEIGHTS serves ntiles matmuls."""
                DR = mybir.MatmulPerfMode.DoubleRow
                for g in range(3):  # pairs: taps (0,g)+(1,g)
                    lhsT = bass.AP(tensor=ws.tensor, offset=ws.offset + 256 * g,
                                   ap=[list(ws.ap[0]), [P, 2], [1, P]])
                    for j in range(ntiles):
                        base = 8 * (t0 + j) * WP + g
                        rhs = bass.AP(tensor=bf.tensor, offset=bf.offset + base,
                                      ap=[list(bf.ap[0]), [WP, 2], [1, TF]])
                        nc.tensor.matmul(
                            ps[:, j, :], lhsT=lhsT, rhs=rhs,
                            start=(g == 0), stop=False, perf_mode=DR)
                for g in range(3):  # single taps (2,g)
                    lhsT = ws[:, 768 + P * g:768 + P * (g + 1)]
                    for j in range(ntiles):
                        off = (8 * (t0 + j) + 2) * WP + g
                        nc.tensor.matmul(
                            ps[:, j, :], lhsT=lhsT, rhs=bf[:, off:off + TF],
                            start=False, stop=(g == 2))

            def xa_load(n):
                """Load x_act for image n (triple-buffered: the slot's last
                reader is conv1(n-3))."""
                nc.sync.dma_start(out=XA[n % 3], in_=xa_ap[n, 0:P])

            def u1(n, nchunks=1):
                """Binarize x_act into padded B1 (DVE; f32 -> fp8)."""
                s = n % 2
                xa = XA[n % 3]
                bounds = {1: ((0, 56),),
                          2: ((0, 26), (26, 56)),
                          4: ((0, 10), (10, 26), (26, 42), (42, 56))}[nchunks]
                with tc.high_priority(offset=150):
                    for r0, r1 in bounds:
                        nc.vector.tensor_scalar(
                            out=B1[s][:, 1 + r0:1 + r1, 1:57], in0=xa[:, r0:r1],
                            scalar1=0.0, scalar2=None, op0=ALU.is_ge)

            def prelude_idle_loads(n):
                """Idle-half loads for image n (must trail conv2(n-2),
                which reads the same A2 slot)."""
                nc.sync.dma_start(out=A2[n % 2][64:128], in_=xa_ap[n, P:P + 64])
                nc.sync.dma_start(out=XH[n % 2][64:128], in_=xh_ap[n, 0:64])

            def conv1(n):
                s = n % 2
                xa, v, a2, fo1, xh = XA[n % 3], V[s], A2[s], FO1[s], XH[s]
                b1f = _flat(B1[s])
                # image 0 uses 2-tile groups so the first matmul only needs
                # 18 rows of x loaded (shorter head critical path)
                groups = ((0, 1), (1, 2), (3, 2), (5, 2)) if n == 0 else ((0, 3), (3, 4))
                for t0, ntiles in groups:
                    yield
                    nr = 8 * ntiles
                    r0 = 8 * t0
                    ps = psum_pool.tile([P, ntiles, 512], F32)
                    t1 = stage_pool.tile([P, nr * W], F32, tag="t1")
                    t13 = t1.rearrange("p (r c) -> p r c", c=W)[:, 0:nr, :]
                    ps3 = _flat(ps).rearrange("p (r c) -> p r c", c=WP)[:, 0:nr, 0:W]
                    emit_conv_group(ps, WS1, b1f, t0, ntiles)
                    # bn apply (f32, same rounding as reference path)
                    nc.scalar.activation(t13, ps3, ACTF.Identity, bias=b1, scale=s1)
                    # pre-clip sum (f32)
                    nc.vector.tensor_tensor(
                        out=v[:, r0:r0 + nr, :], in0=t13, in1=xa[:, r0:r0 + nr, :],
                        op=ALU.add)
                yield
                if n + 1 < IMGS_PER_CORE:
                    u1(n + 1, nchunks=2)
                # hardtanh lo -> A2[0:64] (residual-2 / u2 source), 2 chunks
                # so u2's first half can start a chunk earlier
                with tc.high_priority(offset=150):
                    nc.gpsimd.tensor_scalar(
                        out=a2[0:64, 0:26], in0=v[0:64, 0:26],
                        scalar1=1.0, scalar2=-1.0, op0=ALU.min, op1=ALU.max)
                yield
                with tc.high_priority(offset=150):
                    # u2: binarize [clip(v1)_lo ; x_idle_lo]; per-partition
                    # threshold makes (x >= -move0) == binarize(x+move0)
                    # exact. Rows 1:27 cover everything conv2's first
                    # (3-tile) group reads; the whole chain back to TT1
                    # needs only first-chunk data at every stage.
                    nc.vector.tensor_scalar(
                        out=B2[s][:, 1:27, 1:57], in0=a2[:, 0:26],
                        scalar1=u2thr, scalar2=None, op0=ALU.is_ge)
                    nc.gpsimd.tensor_scalar(
                        out=a2[0:64, 26:56], in0=v[0:64, 26:56],
                        scalar1=1.0, scalar2=-1.0, op0=ALU.min, op1=ALU.max)
                yield
                with tc.high_priority(offset=150):
                    nc.vector.tensor_scalar(
                        out=B2[s][:, 27:57, 1:57], in0=a2[:, 26:56],
                        scalar1=u2thr, scalar2=None, op0=ALU.is_ge)
            def ch13_out(n):
                """Output-only channel groups 1/3 for image n; emitted
                inside conv2(n) so they stay off the image-transition
                critical path (conv2 writes V2, so V stays readable)."""
                s = n % 2
                v, fo1, xh = V[s], FO1[s], XH[s]
                # output-only: negative priority so the list scheduler never
                # hoists these ahead of the binarize chains feeding the PE
                with tc.high_priority(offset=-250):
                    # out ch1 = clip(out1_hi) + move1_even, computed as
                    # clamp(v+b, b-1, b+1) (exact, saturation rounds the
                    # same). DVE-only chain: no cross-engine stall can ever
                    # head-of-line-block the DVE queue here.
                    nc.vector.tensor_scalar(
                        out=fo1[64:128], in0=v[64:128],
                        scalar1=beta_hi, scalar2=beta_p1,
                        op0=ALU.add, op1=ALU.min)
                    nc.vector.tensor_scalar(
                        out=fo1[64:128], in0=fo1[64:128],
                        scalar1=beta_m1, scalar2=None, op0=ALU.max)
                    nc.scalar.dma_start(out=out_ch4(n, 1), in_=_flat(fo1)[64:128])
                    # out ch3 = x_idle_hi + (move0_hi+move1_odd), in place
                    nc.scalar.activation(xh[64:128], xh[64:128], ACTF.Identity,
                                         bias=cxh, scale=1.0)
                    nc.scalar.dma_start(out=out_ch4(n, 3), in_=_flat(xh)[64:128])

            def conv2(n):
                s = n % 2
                a2, v, ot2 = A2[s], V2[s], OT2[s]
                b2f = _flat(B2[s])
                final = n == IMGS_PER_CORE - 1
                for gi, (t0, ntiles) in enumerate(((0, 3), (3, 4))):
                    yield
                    nr = 8 * ntiles
                    r0 = 8 * t0
                    ps = psum_pool.tile([P, ntiles, 512], F32)
                    t2 = stage_pool.tile([P, nr * W], F32, tag="t2")
                    t23 = t2.rearrange("p (r c) -> p r c", c=W)
                    ps3 = _flat(ps).rearrange("p (r c) -> p r c", c=WP)[:, :, 0:W]
                    emit_conv_group(ps, WS2, b2f, t0, ntiles)
                    if gi == 1:
                        ch13_out(n)
                    # bn apply + residual (lo: clipped out1; hi: raw
                    # x_idle_lo, bias folded); the final image slices finer
                    # so the drain chain after the last matmul is short.
                    slices = ((0, nr),) if not (final and gi == 1) else ((0, 24), (24, nr))
                    for a, b in slices:
                        nc.scalar.activation(t23[:, a:b, :], ps3[:, a:b, :],
                                             ACTF.Identity, bias=b2, scale=s2)
                        nc.vector.tensor_tensor(
                            out=v[:, r0 + a:r0 + b, :], in0=t23[:, a:b, :],
                            in1=a2[:, r0 + a:r0 + b, :], op=ALU.add)
                    if gi == 0:
                        # first-part output as soon as rows 0:24 exist
                        nc.gpsimd.tensor_scalar(
                            out=ot2[:, 0:24], in0=v[:, 0:24],
                            scalar1=1.0, scalar2=-1.0, op0=ALU.min, op1=ALU.max)
                        nc.sync.dma_start(out=out_ch4(n, 0, 0, 24),
                                          in_=_flat(ot2)[0:64, 0:24 * W])
                        nc.sync.dma_start(out=out_ch4(n, 2, 0, 24),
                                          in_=_flat(ot2)[64:128, 0:24 * W])
                yield
                hw = 24 * W
                if final:
                    nc.gpsimd.tensor_scalar(
                        out=ot2[:, 24:48], in0=v[:, 24:48],
                        scalar1=1.0, scalar2=-1.0, op0=ALU.min, op1=ALU.max)
                    nc.sync.dma_start(out=out_ch4(n, 0, 24, 48), in_=_flat(ot2)[0:64, hw:48 * W])
                    nc.sync.dma_start(out=out_ch4(n, 2, 24, 48), in_=_flat(ot2)[64:128, hw:48 * W])
                    nc.gpsimd.tensor_scalar(
                        out=ot2[:, 48:56], in0=v[:, 48:56],
                        scalar1=1.0, scalar2=-1.0, op0=ALU.min, op1=ALU.max)
                    nc.sync.dma_start(out=out_ch4(n, 0, 48, 56), in_=_flat(ot2)[0:64, 48 * W:H * W])
                    nc.sync.dma_start(out=out_ch4(n, 2, 48, 56), in_=_flat(ot2)[64:128, 48 * W:H * W])
                else:
                    nc.gpsimd.tensor_scalar(
                        out=ot2[:, 24:56], in0=v[:, 24:56],
                        scalar1=1.0, scalar2=-1.0, op0=ALU.min, op1=ALU.max)
                    nc.sync.dma_start(out=out_ch4(n, 0, 24, 56), in_=_flat(ot2)[0:64, hw:H * W])
                    nc.sync.dma_start(out=out_ch4(n, 2, 24, 56), in_=_flat(ot2)[64:128, hw:H * W])

            # software pipeline across images: conv1(n+1) is emitted before
            # conv2(n) so the PE never stalls on the u2(n) dependency chain.
            def run_all(gen):
                if gen is not None:
                    for _ in gen:
                        pass

            # ~3.5us of dummy matmuls on garbage SBUF keeps the PE busy
            # through the HAM SHORT window while the head DMAs land, so the
            # real matmul stream starts at full clock. Results land in a
            # PSUM buffer that is cleared (start=True) before first use.
            ps = psum_pool.tile([P, 4, 512], F32)
            warm_lhsT = _flat(B1[1])[:, 0:P]
            warm_rhs = _flat(B1[1])[:, 0:512]
            for _ in range(12):
                nc.tensor.matmul(ps[:, 0, :], lhsT=warm_lhsT, rhs=warm_rhs,
                                 start=True, stop=True)
            xa_load(1)
            prelude_idle_loads(0)
            xa_load(2)
            prelude_idle_loads(1)
            u1(0, nchunks=4)
            with nc.named_scope("c1_0"):
                run_all(conv1(0))          # embeds u1(1)
            xa_load(3)                     # XA slot 0 free once c1_0 emitted
            g_prev = conv1(1)              # embeds u1(2)
            for n in range(IMGS_PER_CORE):
                if n >= 1 and n + 1 < IMGS_PER_CORE:
                    prelude_idle_loads(n + 1)
                g_c2 = conv2(n)
                if n + 2 < IMGS_PER_CORE:
                    g_next = conv1(n + 2)
                else:
                    g_next = None
                if g_prev is not None:
                    with nc.named_scope(f"c1_{n + 1}"):
                        run_all(g_prev)
                with nc.named_scope(f"c2_{n}"):
                    run_all(g_c2)
                g_prev = g_next

    nc.compile()
    return nc


def _host_prep(w1, w2, bn1_gamma, bn1_beta, bn1_mean, bn1_var,
               bn2_gamma, bn2_beta, bn2_mean, bn2_var, move0_bias, move1_bias):
    f8 = np.float64
    bw1 = np.where(w1 >= 0, 1.0, -1.0).astype(f8)   # [co, ci, 3, 3]
    bw2 = np.where(w2 >= 0, 1.0, -1.0).astype(f8)

    def wlayout(bw):
        # [ci, 1152]: 3 DoubleRow groups (taps (0,g),(1,g)) then 3 singles
        # (taps (2,g)); within a group the two taps' [ci, co] blocks are
        # adjacent (matching the lhsT [K, 2, M] access pattern).
        m = np.zeros((P, 9 * P), np.float64)
        t = bw.transpose(2, 3, 1, 0)  # [ky, kx, ci, co]
        for g in range(3):
            m[:, 256 * g:256 * g + 128] = t[0, g]
            m[:, 256 * g + 128:256 * g + 256] = t[1, g]
            m[:, 768 + 128 * g:768 + 128 * (g + 1)] = t[2, g]
        return np.ascontiguousarray(m).astype(ml_dtypes.float8_e4m3)

    w1m = wlayout(bw1)

    # conv2 channel permutation (both in and out sides)
    pidx = np.arange(P)
    chan = np.where(pidx < 64, 2 * pidx, 2 * (pidx - 64) + 1)  # partition -> x_act2 channel
    bw2p = bw2[np.ix_(chan, chan)]                  # [co', ci', 3, 3]
    w2m = wlayout(bw2p)

    # u-domain: conv_sign = 2*conv_u - c0, c0 = sum of signed weights
    inv1 = bn1_gamma.astype(f8) / np.sqrt(bn1_var.astype(f8) + EPS)
    c0_1 = bw1.sum(axis=(1, 2, 3))
    s1 = 2.0 * inv1
    b1 = bn1_beta.astype(f8) - bn1_mean.astype(f8) * inv1 - inv1 * c0_1

    inv2 = (bn2_gamma.astype(f8) / np.sqrt(bn2_var.astype(f8) + EPS))[chan]
    c0_2 = bw2.sum(axis=(1, 2, 3))[chan]
    s2 = 2.0 * inv2
    b2 = bn2_beta.astype(f8)[chan] - bn2_mean.astype(f8)[chan] * inv2 - inv2 * c0_2
    # fold move0 (idle-lo residual bias) into conv2's bias on hi partitions
    b2 = b2.copy()
    b2[64:128] += move0_bias[np.arange(64)]

    cst = np.zeros((P, 16), np.float64)
    cst[:, 0] = s1
    cst[:, 1] = b1
    cst[:, 2] = s2
    cst[:, 3] = b2
    i = np.arange(64)
    cst[64:128, 4] = move1_bias[2 * i]
    cst[64:128, 6] = np.float32(np.float32(move1_bias[2 * i]) + np.float32(1.0))
    cst[64:128, 7] = np.float32(np.float32(move1_bias[2 * i]) - np.float32(1.0))
    cst[64:128, 5] = -move0_bias[i]
    cst[64:128, 8] = move0_bias[64 + i] + move1_bias[2 * i + 1]
    return w1m, w2m, cst.astype(np.float32)


def kernel(x, w1, w2, bn1_gamma, bn1_beta, bn1_mean, bn1_var,
           bn2_gamma, bn2_beta, bn2_mean, bn2_var, move0_bias, move1_bias,
           _trace=False):
    x = np.asarray(x, np.float32)
    args = [np.asarray(a, np.float32) for a in (
        w1, w2, bn1_gamma, bn1_beta, bn1_mean, bn1_var,
        bn2_gamma, bn2_beta, bn2_mean, bn2_var, move0_bias, move1_bias)]
    w1m, w2m, cst = _host_prep(*args)

    if "nc" not in _CACHE:
        _CACHE["nc"] = _build()
    nc = _CACHE["nc"]

    in_maps = []
    for c in range(NCORES):
        xc = x[IMGS_PER_CORE * c:IMGS_PER_CORE * (c + 1)]
        in_maps.append({
            "xs32": np.ascontiguousarray(xc[:, 0:192]),
            "xs16": np.ascontiguousarray(xc[:, 192:256]).astype(ml_dtypes.bfloat16),
            "w1m": w1m, "w2m": w2m, "cst": cst,
        })
    kw = {}
    if _trace:
        kw = dict(trace=True, trace_kwargs={"title": "basicblock"})
    res = bass_utils.run_bass_kernel_spmd(nc, in_maps, core_ids=list(range(NCORES)), **kw)
    out = np.concatenate(
        [res.results[c]["out"].astype(np.float32) for c in range(NCORES)], axis=0)
    if _trace:
        _CACHE["last_results"] = res
    return out
